# revision 22
# baseline (speedup 1.0000x reference)
"""Block-diagonal grouped GEMM (GroupLinear) on 8 TRN2 NeuronCores.

Problem: x [8, 2048, 4096] f32, W [4096, 4096] f32 where only the 64
diagonal 64x64 blocks of W are used:
    y[b,s, g*64+o] = sum_i x[b,s, g*64+i] * W[g*64+o, g*64+i]

The kernel is HBM-bandwidth bound (per-NC sustained ~420 GB/s; every x
element is read once, every y element written once). The correctness
budget (rel err < 2e-2) is far looser than f16 rounding (~3e-4), so all
device traffic is f16: 16MB x-in + 16MB y-out + 0.5MB weights per core
instead of the 66MB an f32 kernel moves -> ~2.3x.

Strategy:
  - Data-parallel over batch: core b handles x[b] (2048 tokens).
  - Host packs x[b].T into strip-major layout xp [128, 32*2048] f16 so
    every load is one contiguous 0.5-1MB DMA (4-8KB per partition line).
  - Two 64-ch groups pack into one 128-wide block-diagonal weight strip
    [128i, 128o]. Only the compact diagonal blocks (512KB) cross HBM;
    zero-padded strips are expanded on-device (gpsimd memset + strided
    DVE copies) with the first NHEAD strips in their own tiles so
    matmul 0 isn't gated on the full expansion.
  - Per chunk (1-2 strips): load, matmuls [K=128]x[128,512] into 2-bank
    [128,1024] PSUM tiles, one 1024-wide PSUM->SBUF f16 cast per tile
    alternating Vector/Scalar, store.
  - Loads ride the Sync HWDGE ring, stores the Scalar ring, and all
    loads are emitted before any compute/store so a store waiting on
    compute semaphores can never block a later load (HWDGE DMAs are
    FIFO per issuing engine). The two leading chunks are single-strip
    so compute starts early; the last four chunks store in 512KB halves
    on alternating rings to shorten the drain tail.
  - Host unpacks/upcasts y. All device DMAs are perfectly contiguous.
"""

import numpy as np

import concourse.bacc as bacc
import concourse.mybir as mybir
from concourse.tile import TileContext
from concourse.bass_utils import run_bass_kernel_spmd

B, S, C = 8, 2048, 4096
G, GS = 64, 64            # groups, group size (=in_scale=out_scale)
NSTRIP = C // 128         # 32 strips of 128 channels (2 groups each)
TOK = 512                 # matmul moving free dim (PSUM bank = 512 f32)
PB = 1024                 # psum tile width (2 banks), one copy per tile
F16 = mybir.dt.float16
FP32 = mybir.dt.float32

# (start_strip, n_strips) chunks: two single-strip leaders for a short
# pipeline fill, then 1MB double-strip chunks.
CHUNKS = [(0, 1), (1, 1)] + [(c, 2) for c in range(2, NSTRIP, 2)]


NHEAD = 4                 # strips whose weights expand first (small, fast)


def _build_program():
    nc = bacc.Bacc()
    xp = nc.declare_dram_parameter("xp", [128, NSTRIP * S], F16, isOutput=False)
    # Compact diagonal blocks only (512KB instead of the 1MB padded
    # block-diagonal layout): wc[0:64, c, :] = Wdiag[2c].T,
    # wc[64:128, c, :] = Wdiag[2c+1].T.
    wc = nc.declare_dram_parameter("wc", [128, NSTRIP, 64], F16, isOutput=False)
    yp = nc.declare_dram_parameter("yp", [128, NSTRIP * S], F16, isOutput=True)

    with TileContext(nc) as tc:
        with (
            tc.tile_pool(name="wpool", bufs=1) as wpool,
            tc.tile_pool(name="xpool", bufs=5) as xpool,
            tc.tile_pool(name="opool", bufs=3) as opool,
            tc.tile_pool(name="ppool", bufs=4, space="PSUM") as ppool,
        ):
            # Compact weights ride the Scalar (store) ring, which is idle
            # during fill, then expand on-device into zero-padded
            # block-diagonal strips. Head strips get their own tiles so
            # matmul 0 isn't gated on the full expansion.
            wc_sb = wpool.tile([128, NSTRIP, 64], F16)
            nc.scalar.dma_start(
                out=wc_sb[:, :NHEAD, :], in_=wc[:, :NHEAD, :]
            )
            nc.scalar.dma_start(
                out=wc_sb[:, NHEAD:, :], in_=wc[:, NHEAD:, :]
            )
            w_head = wpool.tile([128, NHEAD, 128], F16)
            w_tail = wpool.tile([128, NSTRIP - NHEAD, 128], F16)
            nc.gpsimd.memset(w_head[:], 0.0)
            nc.vector.tensor_copy(
                out=w_head[0:64, :, 0:64], in_=wc_sb[0:64, :NHEAD, :]
            )
            nc.vector.tensor_copy(
                out=w_head[64:128, :, 64:128], in_=wc_sb[64:128, :NHEAD, :]
            )
            nc.gpsimd.memset(w_tail[:], 0.0)
            nc.vector.tensor_copy(
                out=w_tail[0:64, :, 0:64], in_=wc_sb[0:64, NHEAD:, :]
            )
            nc.vector.tensor_copy(
                out=w_tail[64:128, :, 64:128], in_=wc_sb[64:128, NHEAD:, :]
            )

            def w_strip(c):
                if c < NHEAD:
                    return w_head[:, c, :]
                return w_tail[:, c - NHEAD, :]

            # Emit every load first: each ring's queue is then all loads
            # (paced by xpool buffer reuse), so a drain-phase store issued
            # later can never block a load (HWDGE DMAs are FIFO per
            # issuing engine). Chunks 3-4 load on the Scalar ring: during
            # the pipeline fill no stores exist yet, so that ring would
            # otherwise idle after the 0.5MB weight load.
            x_tiles = []
            for ci, (c0, ns) in enumerate(CHUNKS):
                x_t = xpool.tile([128, ns * S], F16)
                eng = nc.scalar if ci in (3, 4) else nc.sync
                eng.dma_start(
                    out=x_t[:], in_=xp[:, c0 * S : c0 * S + ns * S]
                )
                x_tiles.append(x_t)

            ncopy = 0
            for ci, (c0, ns) in enumerate(CHUNKS):
                cw = ns * S
                x_t = x_tiles[ci]
                o_t = opool.tile([128, cw], F16)
                for pb in range(cw // PB):
                    s, half = divmod(pb, 2)
                    ps = ppool.tile([128, PB], FP32)
                    for q in range(PB // TOK):
                        off = s * S + half * PB + q * TOK
                        nc.tensor.matmul(
                            out=ps[:, q * TOK : (q + 1) * TOK],
                            lhsT=w_strip(c0 + s),
                            rhs=x_t[:, off : off + TOK],
                            start=True,
                            stop=True,
                        )
                    dst = o_t[:, pb * PB : (pb + 1) * PB]
                    if ncopy % 2 == 0:
                        nc.vector.tensor_copy(out=dst, in_=ps[:])
                    else:
                        nc.scalar.copy(out=dst, in_=ps[:])
                    ncopy += 1
                if ci >= len(CHUNKS) - 4:
                    # Drain: store each 512KB half as soon as its copies
                    # land, on alternating rings (loads are done; Sync
                    # ring is idle).
                    h = cw // 2
                    nc.sync.dma_start(
                        out=yp[:, c0 * S : c0 * S + h], in_=o_t[:, :h]
                    )
                    nc.scalar.dma_start(
                        out=yp[:, c0 * S + h : c0 * S + cw], in_=o_t[:, h:]
                    )
                else:
                    nc.scalar.dma_start(
                        out=yp[:, c0 * S : c0 * S + cw], in_=o_t[:]
                    )
    nc.finalize()
    return nc


def _prep_in_maps(x, W):
    # Diagonal blocks: Wdiag[g][o, i] = W[g*64+o, g*64+i]
    Wr = W.reshape(G, GS, G, GS)
    g = np.arange(G)
    WdT = Wr[g, :, g, :].transpose(0, 2, 1).astype(np.float16)    # [g, i, o]
    wc = np.empty((128, NSTRIP, 64), dtype=np.float16)
    wc[0:64] = WdT[0::2].transpose(1, 0, 2)      # [i, c, o] for even groups
    wc[64:128] = WdT[1::2].transpose(1, 0, 2)    # odd groups
    wc = np.ascontiguousarray(wc)
    maps = []
    for b in range(B):
        # xp[p, c*S + t] = x[b, t, c*128 + p]
        xp = np.ascontiguousarray(
            x[b].T.reshape(NSTRIP, 128, S).transpose(1, 0, 2).reshape(128, NSTRIP * S),
            dtype=np.float16,
        )
        maps.append({"xp": xp, "wc": wc})
    return maps


def run(x, W, trace=False, **kw):
    x = np.asarray(x, dtype=np.float32)
    W = np.asarray(W, dtype=np.float32)
    nc = _build_program()
    in_maps = _prep_in_maps(x, W)
    res = run_bass_kernel_spmd(nc, in_maps, list(range(B)), trace=trace, **kw)
    y = np.empty((B, S, C), dtype=np.float32)
    for b in range(B):
        yp = res.results[b]["yp"]
        # y[b, t, c*128 + p] = yp[p, c*S + t]
        y[b] = (
            yp.reshape(128, NSTRIP, S)
            .transpose(1, 0, 2)
            .reshape(C, S)
            .T.astype(np.float32)
        )
    return y, res


def kernel(x, W):
    y, _ = run(x, W, trace=False)
    return y


# revision 24
# speedup vs baseline: 1.1395x; 1.1395x over previous
"""Block-diagonal grouped GEMM (GroupLinear) on 8 TRN2 NeuronCores.

Problem: x [8, 2048, 4096] f32, W [4096, 4096] f32 where only the 64
diagonal 64x64 blocks of W are used:
    y[b,s, g*64+o] = sum_i x[b,s, g*64+i] * W[g*64+o, g*64+i]

The kernel is HBM-bandwidth bound (per-NC sustained ~420 GB/s; every x
element is read once, every y element written once). The correctness
budget (rel err < 2e-2) is far looser than f16 rounding (~3e-4), so all
device traffic is f16: 16MB x-in + 16MB y-out + 0.5MB weights per core
instead of the 66MB an f32 kernel moves -> ~2.3x.

Strategy:
  - Data-parallel over batch: core b handles x[b] (2048 tokens).
  - Host packs x[b].T into strip-major layout xp [128, 32*2048] f16 so
    every load is one contiguous 0.5-1MB DMA (4-8KB per partition line).
  - Two 64-ch groups pack into one 128-wide block-diagonal weight strip
    [128i, 128o]. Only the compact diagonal blocks (512KB) cross HBM;
    zero-padded strips are expanded on-device (gpsimd memset + strided
    DVE copies) with the first NHEAD strips in their own tiles so
    matmul 0 isn't gated on the full expansion.
  - Per chunk (1-2 strips): load, matmuls [K=128]x[128,512] into 2-bank
    [128,1024] PSUM tiles, one 1024-wide PSUM->SBUF f16 cast per tile
    alternating Vector/Scalar, store.
  - Loads ride the Sync HWDGE ring, stores the Scalar ring, and all
    loads are emitted before any compute/store so a store waiting on
    compute semaphores can never block a later load (HWDGE DMAs are
    FIFO per issuing engine). The two leading chunks are single-strip
    so compute starts early; the last four chunks store in 512KB halves
    on alternating rings to shorten the drain tail.
  - Host unpacks/upcasts y. All device DMAs are perfectly contiguous.
"""

import numpy as np

import concourse.bacc as bacc
import concourse.mybir as mybir
from concourse.tile import TileContext
from concourse.bass_utils import run_bass_kernel_spmd

B, S, C = 8, 2048, 4096
G, GS = 64, 64            # groups, group size (=in_scale=out_scale)
NSTRIP = C // 128         # 32 strips of 128 channels (2 groups each)
TOK = 512                 # matmul moving free dim (PSUM bank = 512 f32)
PB = 1024                 # psum tile width (2 banks), one copy per tile
F16 = mybir.dt.float16
FP32 = mybir.dt.float32

# (start_strip, n_strips) chunks: two single-strip leaders for a short
# pipeline fill, then 1MB double-strip chunks.
CHUNKS = [(0, 1), (1, 1)] + [(c, 2) for c in range(2, NSTRIP, 2)]


NHEAD = 4                 # strips whose weights expand first (small, fast)


def _build_program():
    nc = bacc.Bacc()
    xp = nc.declare_dram_parameter("xp", [128, NSTRIP * S], F16, isOutput=False)
    # Compact diagonal blocks only (512KB instead of the 1MB padded
    # block-diagonal layout): wc[0:64, c, :] = Wdiag[2c].T,
    # wc[64:128, c, :] = Wdiag[2c+1].T.
    wc = nc.declare_dram_parameter("wc", [128, NSTRIP, 64], F16, isOutput=False)
    yp = nc.declare_dram_parameter("yp", [128, NSTRIP * S], F16, isOutput=True)

    with TileContext(nc) as tc:
        with (
            tc.tile_pool(name="wpool", bufs=1) as wpool,
            tc.tile_pool(name="xpool", bufs=3) as xpool,
            tc.tile_pool(name="opool", bufs=3) as opool,
            tc.tile_pool(name="ppool", bufs=4, space="PSUM") as ppool,
        ):
            # Compact weights ride the Scalar (store) ring, which is idle
            # during fill, then expand on-device into zero-padded
            # block-diagonal strips. Head strips get their own tiles so
            # matmul 0 isn't gated on the full expansion.
            wc_sb = wpool.tile([128, NSTRIP, 64], F16)
            nc.scalar.dma_start(
                out=wc_sb[:, :NHEAD, :], in_=wc[:, :NHEAD, :]
            )
            nc.scalar.dma_start(
                out=wc_sb[:, NHEAD:, :], in_=wc[:, NHEAD:, :]
            )
            w_head = wpool.tile([128, NHEAD, 128], F16)
            w_tail = wpool.tile([128, NSTRIP - NHEAD, 128], F16)
            nc.gpsimd.memset(w_head[:], 0.0)
            nc.vector.tensor_copy(
                out=w_head[0:64, :, 0:64], in_=wc_sb[0:64, :NHEAD, :]
            )
            nc.vector.tensor_copy(
                out=w_head[64:128, :, 64:128], in_=wc_sb[64:128, :NHEAD, :]
            )
            nc.gpsimd.memset(w_tail[:], 0.0)
            nc.vector.tensor_copy(
                out=w_tail[0:64, :, 0:64], in_=wc_sb[0:64, NHEAD:, :]
            )
            nc.vector.tensor_copy(
                out=w_tail[64:128, :, 64:128], in_=wc_sb[64:128, NHEAD:, :]
            )

            def w_strip(c):
                if c < NHEAD:
                    return w_head[:, c, :]
                return w_tail[:, c - NHEAD, :]

            # Emit every load first: the Sync engine's queue is then all
            # loads (paced by xpool buffer reuse), so a drain-phase store
            # issued on Sync can never block a later load (HWDGE DMAs are
            # FIFO per issuing engine). Loads must NOT ride the Scalar
            # (store) ring: anything queued there delays the whole store
            # stream, which sets the drain tail (measured +10us).
            x_tiles = []
            for ci, (c0, ns) in enumerate(CHUNKS):
                x_t = xpool.tile([128, ns * S], F16)
                nc.sync.dma_start(
                    out=x_t[:], in_=xp[:, c0 * S : c0 * S + ns * S]
                )
                x_tiles.append(x_t)

            ncopy = 0
            for ci, (c0, ns) in enumerate(CHUNKS):
                cw = ns * S
                x_t = x_tiles[ci]
                o_t = opool.tile([128, cw], F16)
                for pb in range(cw // PB):
                    s, half = divmod(pb, 2)
                    ps = ppool.tile([128, PB], FP32)
                    for q in range(PB // TOK):
                        off = s * S + half * PB + q * TOK
                        nc.tensor.matmul(
                            out=ps[:, q * TOK : (q + 1) * TOK],
                            lhsT=w_strip(c0 + s),
                            rhs=x_t[:, off : off + TOK],
                            start=True,
                            stop=True,
                        )
                    dst = o_t[:, pb * PB : (pb + 1) * PB]
                    if ncopy % 2 == 0:
                        nc.vector.tensor_copy(out=dst, in_=ps[:])
                    else:
                        nc.scalar.copy(out=dst, in_=ps[:])
                    ncopy += 1
                if ci >= len(CHUNKS) - 4:
                    # Drain: store each 512KB half as soon as its copies
                    # land, on alternating rings (loads are done; Sync
                    # ring is idle).
                    h = cw // 2
                    nc.sync.dma_start(
                        out=yp[:, c0 * S : c0 * S + h], in_=o_t[:, :h]
                    )
                    nc.scalar.dma_start(
                        out=yp[:, c0 * S + h : c0 * S + cw], in_=o_t[:, h:]
                    )
                else:
                    nc.scalar.dma_start(
                        out=yp[:, c0 * S : c0 * S + cw], in_=o_t[:]
                    )
    nc.finalize()
    return nc


def _prep_in_maps(x, W):
    # Diagonal blocks: Wdiag[g][o, i] = W[g*64+o, g*64+i]
    Wr = W.reshape(G, GS, G, GS)
    g = np.arange(G)
    WdT = Wr[g, :, g, :].transpose(0, 2, 1).astype(np.float16)    # [g, i, o]
    wc = np.empty((128, NSTRIP, 64), dtype=np.float16)
    wc[0:64] = WdT[0::2].transpose(1, 0, 2)      # [i, c, o] for even groups
    wc[64:128] = WdT[1::2].transpose(1, 0, 2)    # odd groups
    wc = np.ascontiguousarray(wc)
    maps = []
    for b in range(B):
        # xp[p, c*S + t] = x[b, t, c*128 + p]
        xp = np.ascontiguousarray(
            x[b].T.reshape(NSTRIP, 128, S).transpose(1, 0, 2).reshape(128, NSTRIP * S),
            dtype=np.float16,
        )
        maps.append({"xp": xp, "wc": wc})
    return maps


def run(x, W, trace=False, **kw):
    x = np.asarray(x, dtype=np.float32)
    W = np.asarray(W, dtype=np.float32)
    nc = _build_program()
    in_maps = _prep_in_maps(x, W)
    res = run_bass_kernel_spmd(nc, in_maps, list(range(B)), trace=trace, **kw)
    y = np.empty((B, S, C), dtype=np.float32)
    for b in range(B):
        yp = res.results[b]["yp"]
        # y[b, t, c*128 + p] = yp[p, c*S + t]
        y[b] = (
            yp.reshape(128, NSTRIP, S)
            .transpose(1, 0, 2)
            .reshape(C, S)
            .T.astype(np.float32)
        )
    return y, res


def kernel(x, W):
    y, _ = run(x, W, trace=False)
    return y
